# revision 64
# baseline (speedup 1.0000x reference)
"""Trainium2 Bass kernel for nn_MCMambaLM (MC-Mamba LM, 4 layers, RVQ head).

Self-contained: kernel(**inputs) -> np.ndarray of shape (1, 2048, 8, 1024).

v2 design (vs v1 two-pass scan baseline):
  * Sequence sharding: 8 cores x 256 tokens (= one GRM segment per core).
  * Embedding (RVQ gather+sum) computed host-side, shipped as x0.
  * Single-pass selective scan (zero initial state) + linear correction:
      y = y0 + sum_s C_s * q^{s+1} * h_in[s],  q_t = exp(-cumsum delta)
    so no second scan pass after the cross-core carry AllGather.
  * r = sigmoid(-(dt_pre + dt_b)) = exp(-delta); q = cumprod(r) via one
    scan; dA_s = exp((s+1)*ln r) on the Act engine.
  * Work split across DVE / Pool(gpsimd) / Act; scan internals (dA, b1,
    h, tmp, q, corr) in bf16 (validated ~2.3e-3 absmax-rel err).
  * Collectives: AG1 carry (split a/b for overlap) + AG2 boundary each
    layer; AG0 conv-halo only for layers >= 1 (layer-0 halo from host).
"""

import numpy as np

import concourse.bacc as bacc
import concourse.mybir as mybir
import concourse.tile as tile

F32 = mybir.dt.float32
F32R = mybir.dt.float32r
BF16 = mybir.dt.bfloat16
AOP = mybir.AluOpType
AF = mybir.ActivationFunctionType
AX = mybir.AxisListType

B, T, D = 1, 2048, 768
K, V = 8, 1024
D_IN, D_STATE, D_CONV, DT_RANK = 1536, 16, 4, 48
L, SEG = 4, 256
NC = 8
TL = T // NC            # 256 tokens per core
NB = D_IN // 128        # 12 e-blocks
NBD = D // 128          # 6 d-blocks
W = TL + 1              # 257 = 256 cols + 1 gap
PW = NB * W             # 3084
PWD = NBD * W           # 1542
WE = TL + 3             # 259: 3 halo cols + 256
PWE = NB * WE           # 3108
SCALE = 1.0 / float(np.sqrt(D))
NEG_BIG = -1.0e30
DEBUG = False


def r3(ap, w=W):
    return ap.rearrange("p (g w) -> p g w", w=w)


def zero_ps(nc, ap):
    # zero a PSUM region on the Act engine (then accumulate with start=False)
    nc.scalar.activation(ap, ap, AF.Copy, scale=0.0)


def zero_banks(nc, ident, zeros, ps, nbanks, ofs=0):
    # zero whole PSUM banks on the PE queue itself: start=True resets the
    # bank, and x@0 writes zeros; same-engine ordering vs accumulates.
    for b in range(nbanks):
        nc.tensor.matmul(ps[:, ofs + b * 512:ofs + (b + 1) * 512],
                         ident[:], zeros[:], start=True, stop=False)


def build_module(repeat=1):
    nc = bacc.Bacc("TRN2", target_bir_lowering=False, debug=False,
                   num_devices=NC)

    def inp(name, shape, dt=F32):
        return nc.declare_dram_parameter(name, list(shape), dt, isOutput=False)

    # shared weights
    in_projT = inp("in_projT", [L, 2, NBD, 128, D_IN], F32R)
    in_projTb = inp("in_projTb", [2, 2, NBD, 128, D_IN], BF16)
    out_projT = inp("out_projT", [L, NB, 128, D], F32R)
    out_projTb = inp("out_projTb", [2, NB, 128, D], BF16)
    x_projT = inp("x_projT", [L, NB, 128, 80], F32R)
    dt_wT = inp("dt_wT", [L, DT_RANK, D_IN], BF16)
    W_uT = inp("W_uT", [L, NBD, 128, D], F32R)
    W_uTb = inp("W_uTb", [2, NBD, 128, D], BF16)
    headT = inp("headT", [NBD, 128, K * V], BF16)
    convw_a = inp("convw_a", [L, D_CONV, 128, NB])
    convb_a = inp("convb_a", [L, 128, NB])
    dtb_a = inp("dtb_a", [L, 128, NB])      # = -dt_b
    dskip_a = inp("dskip_a", [L, 128, NB])
    invcnt = inp("invcnt", [128, W])
    ones128 = inp("ones128", [128, 1])
    identB = inp("identB", [128, 128], BF16)
    identF = inp("identF", [128, 128], F32)

    # per-core
    x0_in = inp("x0_in", [128, PWD], F32R)          # embedding, gap layout
    halo0 = inp("halo0", [128, NB * 3])             # layer-0 conv halo
    gsel_h = inp("gsel_h", [128, 8 * 36], BF16)
    mask8 = inp("mask8", [8, 1])
    sel96 = inp("sel96", [128, 96])                 # combine select (j==c-1)

    logits_out = nc.declare_dram_parameter(
        "logits_out", [K * V, TL], F32, isOutput=True)

    with tile.TileContext(nc) as tc:
        with (
            tc.tile_pool(name="cp", bufs=1) as cp,
            tc.tile_pool(name="wp", bufs=2) as wp,
            tc.tile_pool(name="bp", bufs=2) as bp,
            tc.tile_pool(name="sp", bufs=2) as sp,
            tc.tile_pool(name="pp1", bufs=1, space="PSUM") as pp1,
            tc.tile_pool(name="pp2", bufs=2, space="PSUM") as pp2,
            tc.tile_pool(name="dp", bufs=2, space="DRAM") as dp,
        ):
            # ---------- persistent tiles ----------
            def new_xtile(idx, dt_):
                t = cp.tile([128, PWD], dt_, tag=f"xT_{idx % 2}",
                            name=f"xT{idx}")
                m = t[:].bitcast(F32) if dt_ == F32R else t[:]
                nc.vector.memset(m, 0.0)
                return t

            xcur0 = new_xtile(0, F32R)

            us = cp.tile([128, PW], F32R, tag="us")
            nc.vector.memset(us[:].bitcast(F32), 0.0)
            sz = cp.tile([128, PW], F32, tag="sz")
            du = cp.tile([128, PW], BF16, tag="du")
            qq = cp.tile([128, PW], BF16, tag="qq")


            zro512 = cp.tile([128, 512], BF16, tag="zro512")
            nc.vector.memset(zro512[:], 0.0)
            hend = cp.tile([128, 90], F32, tag="hend", name="ag1a_sb")
            hendb = cp.tile([128, 120], BF16, tag="hendb",
                            name="ag1b_sb")
            hin = cp.tile([128, 192], F32, tag="hin")
            ur = cp.tile([128, PWD], F32, tag="ur")
            mst = cp.tile([128, NBD * 8], F32, tag="mst")
            hstk = cp.tile([8, D], F32, tag="hstk")
            dbc_bf = cp.tile([80, W], BF16, tag="dbc_bf")
            nc.vector.memset(dbc_bf[:], 0.0)

            cvw = cp.tile([128, D_CONV * NB], F32, tag="cvw")
            cvb = cp.tile([128, NB], F32, tag="cvb")
            dtb = cp.tile([128, NB], F32, tag="dtb")
            dsk = cp.tile([128, NB], F32, tag="dsk")
            inv_sb = cp.tile([128, W], F32, tag="inv_sb")
            ones128_sb = cp.tile([128, 1], F32, tag="ones128_sb")
            mask8_sb = cp.tile([8, 1], F32, tag="mask8_sb")
            identB_sb = cp.tile([128, 128], BF16, tag="identB_sb")
            identF_sb = cp.tile([128, 128], F32, tag="identF_sb")
            gsel_sb = cp.tile([128, 8 * 36], BF16, tag="gsel_sb")
            sel96_sb = cp.tile([128, 96], F32, tag="sel96_sb")
            halo0_sb = cp.tile([128, NB * 3], F32, tag="halo0_sb")

            nc.sync.dma_start(inv_sb[:], invcnt[:])
            nc.sync.dma_start(ones128_sb[:], ones128[:])
            nc.sync.dma_start(mask8_sb[:], mask8[:])
            nc.sync.dma_start(identB_sb[:], identB[:])
            nc.sync.dma_start(identF_sb[:], identF[:])
            nc.sync.dma_start(gsel_sb[:], gsel_h[:])
            nc.sync.dma_start(sel96_sb[:], sel96[:])
            nc.sync.dma_start(halo0_sb[:], halo0[:])
            nc.sync.dma_start(xcur0[:], x0_in[:])

            # ---------- layers ----------
            xcur = xcur0
            for rep in range(repeat):
                for l in range(L):
                    li = rep * L + l
                    xdt = F32R if li + 1 < 2 else BF16
                    wdt = F32R if li < 2 else BF16
                    xnxt = new_xtile(li + 1, xdt)

                    nc.sync.dma_start(
                        cvw[:].rearrange("p (k g) -> p k g", g=NB),
                        convw_a[l].rearrange("k p g -> p k g"))
                    nc.sync.dma_start(cvb[:], convb_a[l])
                    nc.sync.dma_start(dtb[:], dtb_a[l])
                    nc.sync.dma_start(dsk[:], dskip_a[l])

                    # prologue PSUM instance: 12 chunks of [*,256]
                    prps = pp1.tile([128, NB * TL], F32, tag="bigps",
                                    name="prps")

                    def chunk(i, p=128):
                        return prps[0:p, (i % NB) * TL:((i % NB) + 1) * TL]

                    # ---- GRM early (only needs xcur; keep Pool busy) ----
                    cs = bp.tile([128, PWD], F32, tag="cs", bufs=1, name="cs")
                    xc_f = (xcur[:].bitcast(F32) if li < 2 else xcur[:])
                    nc.vector.memset(du[:, 0:PWD], 1.0)
                    nc.vector.memset(
                        r3(du[:, 0:PWD])[:, :, W - 1], 0.0)
                    nc.vector.tensor_tensor_scan(
                        cs[:], du[:, 0:PWD], xc_f, 0.0,
                        AOP.mult, AOP.add)
                    mrow = sp.tile([128, NBD], F32, tag="mrow", bufs=1)
                    nc.vector.tensor_scalar_mul(
                        mrow[:], r3(cs[:])[:, :, TL - 1], 1.0 / TL)
                    mcur = bp.tile([128, PWD], F32, tag="mcur", bufs=1,
                                   name="mcur")
                    iv = inv_sb[:].unsqueeze(1).to_broadcast((128, NBD, W))
                    nc.gpsimd.tensor_tensor(
                        r3(mcur[:])[:, :, 0:TL], r3(cs[:])[:, :, 0:TL],
                        iv[:, :, 0:TL], AOP.mult)

                    # ---- in_proj ----
                    u_pre = bp.tile([128, PWE], F32, tag="u_pre", bufs=1,
                                    name="u_pre")
                    ph = (pp2.tile([128, 48], F32, tag="pss", name="ph")
                          if li > 0 else None)
                    for half in range(2):
                        zero_banks(nc, identB_sb, zro512, prps, 6)
                        if half == 0 and li > 0:
                            zero_ps(nc, ph[:])
                        for kb in range(NBD):
                            for hw6 in range(2):
                                wt = wp.tile([128, 768], wdt, tag="wmat",
                                             bufs=5, name="wt")
                                wsrc = (in_projT[l, half, kb] if li < 2
                                        else in_projTb[l - 2, half, kb])
                                nc.scalar.dma_start(
                                    wt[:],
                                    wsrc[:, hw6 * 768:(hw6 + 1) * 768])
                                for i in range(6):
                                    m = hw6 * 6 + i
                                    nc.tensor.matmul(
                                        chunk(m),
                                        wt[:, i * 128:(i + 1) * 128],
                                        xcur[:, kb * W:kb * W + TL],
                                        start=False, stop=(kb == NBD - 1))
                                    if half == 0 and li > 0:
                                        nc.tensor.matmul(
                                            ph[:, m * 4:(m + 1) * 4],
                                            wt[:, i * 128:(i + 1) * 128],
                                            xcur[:, kb * W + TL - 4:
                                                 kb * W + TL],
                                            start=False,
                                            stop=(kb == NBD - 1))
                        if half == 0 and li > 0:
                            # own last-3 u columns -> AG0, early
                            ag0s = sp.tile([128, 36], BF16, tag="ag0s")
                            nc.scalar.copy(
                                ag0s[:].rearrange("p (g c) -> p g c", c=3),
                                ph[:].rearrange("p (g c) -> p g c", c=4)
                                [:, :, 1:4])
                            ag0_in = dp.tile([128, 36], BF16, tag="ag0i")
                            ag0_out = dp.tile([NC * 128, 36], BF16,
                                              tag="ag0o")
                            nc.sync.dma_start(ag0_in[:], ag0s[:])
                            nc.gpsimd.collective_compute(
                                "AllGather", AOP.bypass,
                                ins=[ag0_in[:].opt()],
                                outs=[ag0_out[:].opt()],
                                replica_groups=[list(range(NC))])
                        for m in range(12):
                            if half == 0:
                                nc.scalar.copy(
                                    u_pre[:, m * WE + 3:(m + 1) * WE],
                                    chunk(m))
                            else:
                                nc.scalar.activation(
                                    sz[:, m * W:m * W + TL], chunk(m),
                                    AF.Silu)

                    # ---- conv halo ----
                    if li == 0:
                        nc.vector.tensor_copy(
                            r3(u_pre[:], WE)[:, :, 0:3],
                            halo0_sb[:].rearrange("p (g c) -> p g c", c=3))
                    else:
                        halo_r = sp.tile([128, 8 * 36], BF16, tag="halo", bufs=1,
                                         name="halo_r")
                        nc.sync.dma_start(
                            halo_r[:].rearrange("p (j c) -> p j c", j=8),
                            ag0_out[:].rearrange("(j p) c -> p j c", p=128))
                        halo_m = sp.tile([128, 8 * 36], BF16, tag="halom", bufs=1,
                                         name="halo_m")
                        nc.vector.tensor_tensor(
                            halo_m[:], halo_r[:], gsel_sb[:], AOP.mult)
                        halo36 = sp.tile([128, 36], F32, tag="halo36",
                                         bufs=1, name="halo36")
                        nc.vector.tensor_reduce(
                            halo36[:],
                            halo_m[:].rearrange("p (j c) -> p c j", j=8),
                            AX.X, AOP.add)
                        nc.vector.tensor_copy(
                            r3(u_pre[:], WE)[:, :, 0:3], r3(halo36[:], 3))

                    # ---- conv (4-tap causal) + bias + silu -> us ----
                    u3 = r3(u_pre[:], WE)
                    for g in range(NB):
                        uct = bp.tile([128, TL], F32, tag="uct", bufs=2,
                                      name="uct")
                        for kk in range(D_CONV):
                            if kk == 0:
                                nc.vector.tensor_scalar_mul(
                                    uct[:], u3[:, g, 0:TL],
                                    cvw[:, g:g + 1])
                            else:
                                ctt = bp.tile([128, TL], F32, tag="ctt",
                                              bufs=3, name="ctt")
                                nc.vector.tensor_scalar_mul(
                                    ctt[:], u3[:, g, kk:kk + TL],
                                    cvw[:, kk * NB + g:kk * NB + g + 1])
                                add_eng = (nc.vector if g % 2 == 0
                                           else nc.gpsimd)
                                add_eng.tensor_tensor(
                                    uct[:], uct[:], ctt[:], AOP.add)
                        nc.scalar.activation(
                            us[:, g * W:g * W + TL],
                            uct[:], AF.Silu, bias=cvb[:, g:g + 1])

                    # ---- x_proj -> dbc ----
                    psd_t = pp2.tile([80, TL], F32, tag="pss", name="psd")
                    psd = psd_t[:]
                    for kb in range(NB):
                        xpw = wp.tile([128, 80], F32R, tag="wsml", name="xpw")
                        nc.scalar.dma_start(xpw[:], x_projT[l, kb])
                        nc.tensor.matmul(
                            psd, xpw[:], us[:, kb * W:kb * W + TL],
                            start=(kb == 0), stop=(kb == NB - 1))
                    nc.scalar.copy(dbc_bf[:, 0:TL], psd)

                    # ---- dt -> r, lnr, du, q ----
                    rr = bp.tile([128, PW], F32, tag="ygsh", bufs=1,
                                 name="rr")
                    nc.vector.memset(r3(rr[:])[:, :, W - 1], 0.0)
                    dtw = wp.tile([DT_RANK, D_IN], BF16, tag="dtw", bufs=1,
                                  name="dtw")
                    nc.scalar.dma_start(dtw[:], dt_wT[l])
                    for g in range(NB):
                        pst_t = pp2.tile([128, TL], F32, tag="pss",
                                         name="pst")
                        pst = pst_t[:]
                        nc.tensor.matmul(
                            pst, dtw[:, g * 128:(g + 1) * 128],
                            dbc_bf[0:48, 0:TL], start=True, stop=True)
                        # r = sigmoid(-(pre + dt_b)) = exp(-delta)
                        nc.scalar.activation(
                            rr[:, g * W:g * W + TL], pst, AF.Sigmoid,
                            scale=-1.0, bias=dtb[:, g:g + 1])
                    lnr = bp.tile([128, PW], BF16, tag="lnr", bufs=1,
                                  name="lnr")
                    nc.scalar.activation(lnr[:], rr[:], AF.Ln)
                    nc.vector.memset(r3(lnr[:])[:, :, W - 1], NEG_BIG)
                    # q = cumprod(r): gap-seed pattern built in du's buffer
                    nc.vector.memset(du[:], 0.0)
                    nc.vector.memset(r3(du[:])[:, :, W - 1], 1.0)
                    nc.vector.tensor_tensor_scan(
                        qq[:], rr[:], du[:], 1.0, AOP.mult, AOP.add)
                    # du = -lnr * us  (= delta * u), bf16
                    nc.vector.scalar_tensor_tensor(
                        du[:], lnr[:], -1.0, us[:].bitcast(F32),
                        AOP.mult, AOP.mult)
                    nc.vector.memset(r3(du[:])[:, :, W - 1], 0.0)

                    zero_banks(nc, identB_sb, zro512, prps, 3, 6 * TL)
                    for kb in range(NBD):
                        wut = wp.tile([128, 768], wdt, tag="wmat",
                                      bufs=5, name="wut")
                        nc.scalar.dma_start(
                            wut[:],
                            W_uT[l, kb] if li < 2 else W_uTb[l - 2, kb])
                        for m in range(6):
                            nc.tensor.matmul(
                                chunk(6 + m), wut[:, m * 128:(m + 1) * 128],
                                xcur[:, kb * W:kb * W + TL],
                                start=False, stop=(kb == NBD - 1))
                    for m in range(6):
                        nc.scalar.copy(
                            ur[:, m * W:m * W + TL], chunk(6 + m))
                    prodt = bp.tile([128, PWD], F32, tag="prod", bufs=1,
                                    name="prodt")
                    nc.gpsimd.tensor_tensor(
                        r3(prodt[:])[:, :, 0:TL], r3(mcur[:])[:, :, 0:TL],
                        r3(ur[:])[:, :, 0:TL], AOP.mult)
                    ps_cur = pp2.tile([1, TL], F32, tag="pss", name="ps_cur")
                    for m in range(NBD):
                        nc.tensor.matmul(
                            ps_cur[:], ones128_sb[:],
                            prodt[:, m * W:m * W + TL],
                            start=(m == 0), stop=(m == NBD - 1))
                    s_cur = sp.tile([1, TL], F32, tag="s_cur", bufs=1)
                    nc.scalar.copy(s_cur[:], ps_cur[:])

                    # ---- pass-1 scans + y0 accumulation ----
                    # software-pipelined issue order: broadcasts one step
                    # ahead, h->tmp->psy one step behind the scan, so the
                    # in-order DVE queue never blocks on a pending scan.
                    psy = pp1.tile([128, NB * TL], F32, tag="bigps",
                                   name="psy")
                    phb = pp2.tile([128, NB], F32, tag="pss", name="phb")

                    def bcast_row(row, tag, bufs):
                        stg = sp.tile([1, W], BF16, tag="stg", bufs=2,
                                      name="stg")
                        nc.scalar.dma_start(stg[:], dbc_bf[row:row + 1, :])
                        bc = sp.tile([128, W], BF16, tag=tag, bufs=bufs,
                                     name=tag)
                        nc.gpsimd.partition_broadcast(bc[:], stg[:])
                        return bc

                    def drain(s, h1):
                        # emit tmp / hend / psy for state s
                        if s < 6:
                            nc.vector.tensor_copy(
                                hend[:, 18 + s * NB:18 + (s + 1) * NB],
                                r3(h1[:])[:, :, TL - 1])
                        else:
                            nc.vector.tensor_copy(
                                hendb[:, (s - 6) * NB:(s - 5) * NB],
                                r3(h1[:])[:, :, TL - 1])
                        tmp = bp.tile([128, NB * TL], BF16, tag="b1",
                                      bufs=3, name="tmp")
                        tmp3 = tmp[:].rearrange("p (g t) -> p g t", t=TL)
                        cb = Cbcs[s][:].unsqueeze(1).to_broadcast(
                            (128, NB, W))
                        tmp_eng = nc.gpsimd if s % 3 == 2 else nc.vector
                        tmp_eng.tensor_tensor(
                            tmp3, r3(h1[:])[:, :, 0:TL], cb[:, :, 0:TL],
                            AOP.mult)
                        for b in range(6):
                            nc.tensor.matmul(
                                psy[:, b * 512:(b + 1) * 512], identB_sb[:],
                                tmp[:, b * 512:(b + 1) * 512],
                                start=(s == 0), stop=False)
                        nc.tensor.matmul(
                            phb[:], identB_sb[:],
                            tmp3[:, :, TL - 1],
                            start=(s == 0), stop=False)
                        if s == 5:
                            nc.vector.tensor_copy(
                                hend[:, 0:NB], r3(qq[:])[:, :, TL - 1])
                            nc.vector.tensor_copy(hend[:, 12:18], mrow[:])
                            ag1a_in = dp.tile([128, 90], F32, tag="ag1ai")
                            ag1a_out_l = dp.tile([NC * 128, 90], F32,
                                                 tag="ag1ao")
                            nc.sync.dma_start(ag1a_in[:], hend[:])
                            nc.gpsimd.collective_compute(
                                "AllGather", AOP.bypass,
                                ins=[ag1a_in[:].opt()],
                                outs=[ag1a_out_l[:].opt()],
                                replica_groups=[list(range(NC))])
                            return ag1a_out_l
                        return None

                    Bbcs, Cbcs = {}, {}
                    Bbcs[0] = bcast_row(48, "Bbc", 3)
                    Cbcs[0] = bcast_row(64, "Cbc", 3)
                    pend = None
                    ag1a_out = None
                    for s in range(D_STATE):
                        if s + 1 < D_STATE:
                            Bbcs[s + 1] = bcast_row(48 + s + 1, "Bbc", 3)
                            Cbcs[s + 1] = bcast_row(64 + s + 1, "Cbc", 3)
                        dA = bp.tile([128, PW], BF16, tag="dA", bufs=3,
                                     name="dA")
                        nc.scalar.activation(dA[:], lnr[:], AF.Exp,
                                             scale=float(s + 1))
                        b1 = bp.tile([128, PW], BF16, tag="b1", bufs=3,
                                     name="b1")
                        bb = Bbcs[s][:].unsqueeze(1).to_broadcast(
                            (128, NB, W))
                        h1 = bp.tile([128, PW], BF16, tag="dA", bufs=3,
                                     name="h1")
                        b1_eng = nc.gpsimd if s % 3 == 1 else nc.vector
                        b1_eng.tensor_tensor(
                            r3(b1[:]), r3(du[:]), bb, AOP.mult)
                        nc.vector.tensor_tensor_scan(
                            h1[:], dA[:], b1[:], 0.0, AOP.mult, AOP.add)
                        if pend is not None:
                            r = drain(pend[0], pend[1])
                            if r is not None:
                                ag1a_out = r
                        pend = (s, h1)
                    r = drain(pend[0], pend[1])
                    if r is not None:
                        ag1a_out = r

                    # AG1b
                    ag1b_in = dp.tile([128, 120], BF16, tag="ag1bi")
                    ag1b_out = dp.tile([NC * 128, 120], BF16, tag="ag1bo")
                    nc.sync.dma_start(ag1b_in[:], hendb[:])
                    nc.gpsimd.collective_compute(
                        "AllGather", AOP.bypass,
                        ins=[ag1b_in[:].opt()], outs=[ag1b_out[:].opt()],
                        replica_groups=[list(range(NC))])

                    # ---- AG1a processing: combine carry for s=0..7 ----
                    agav = ag1a_out[:].rearrange("(j p) c -> p j c", p=128)
                    qe_st = sp.tile([128, 96], F32, tag="qe_st", bufs=1,
                                    name="qe_st")
                    nc.sync.dma_start(
                        qe_st[:].rearrange("p (j g) -> p j g", g=NB),
                        agav[:, :, 0:NB])
                    qe_all = sp.tile([128, 96], F32, tag="qe_all", bufs=1,
                                     name="qe_all")
                    nc.vector.tensor_copy(
                        qe_all[:].rearrange("p (g j) -> p g j", j=8),
                        qe_st[:].rearrange("p (j g) -> p g j", g=NB))
                    for g in range(NBD):
                        nc.sync.dma_start(
                            mst[:].bitcast(F32)[:, g * 8:(g + 1) * 8],
                            agav[:, :, NB + g])
                    he_st = sp.tile([128, 8 * 72], F32, tag="he_st", bufs=1,
                                    name="he_st")
                    nc.sync.dma_start(
                        he_st[:].rearrange("p (j sg) -> p j sg", j=8),
                        agav[:, :, 18:90])
                    he_a = sp.tile([128, 8 * 72], F32, tag="he_a", bufs=1,
                                   name="he_a")
                    nc.vector.tensor_copy(
                        he_a[:].rearrange("p (sg j) -> p sg j", j=8),
                        he_st[:].rearrange("p (j sg) -> p sg j", j=8))
                    nc.vector.memset(
                        qe_all[:].rearrange("p (g j) -> p g j", j=8)
                        [:, :, 0:1], 0.0)
                    qp = sp.tile([128, 96], F32, tag="qp", bufs=2,
                                 name="qp")
                    nc.vector.tensor_copy(qp[:], qe_all[:])

                    def combine(s, he_tile, sofs, qp_t):
                        Hs = sp.tile([128, 96], F32, tag="Hs", bufs=1,
                                     name="Hs")
                        nc.vector.tensor_tensor_scan(
                            Hs[:], qp_t[:],
                            he_tile[:, sofs * 96:(sofs + 1) * 96],
                            0.0, AOP.mult, AOP.add)
                        Hm = sp.tile([128, 96], F32, tag="Hm", bufs=1,
                                     name="Hm")
                        nc.vector.tensor_tensor(
                            Hm[:], Hs[:], sel96_sb[:], AOP.mult)
                        nc.vector.tensor_reduce(
                            hin[:, s * NB:(s + 1) * NB],
                            Hm[:].rearrange("p (g j) -> p g j", j=8),
                            AX.X, AOP.add)
                        if s < D_STATE - 1:
                            qp2 = sp.tile([128, 96], F32, tag="qp", bufs=2,
                                          name="qp2")
                            nc.gpsimd.tensor_tensor(
                                qp2[:], qp_t[:], qe_all[:], AOP.mult)
                            return qp2
                        return qp_t

                    for s in range(6):
                        qp = combine(s, he_a, s, qp)

                    # ---- corr: p chain + C mult + hin scale into psy ----
                    # truncated to the decay window; s=0 done LAST at full
                    # width so the stop=True matmuls cover whole regions.
                    def kof(s):
                        return min(TL, 768 // (s + 1))

                    pq = qq

                    def corr(s, pq_t):
                        k = kof(s)
                        stgC2 = sp.tile([1, W], BF16, tag="stg", bufs=2,
                                        name="stgC2")
                        nc.scalar.dma_start(stgC2[:],
                                            dbc_bf[64 + s:65 + s, :])
                        Cb2 = sp.tile([128, W], BF16, tag="Cbc", bufs=3,
                                      name="Cb2")
                        nc.gpsimd.partition_broadcast(Cb2[:], stgC2[:])
                        pc = bp.tile([128, NB * TL], BF16, tag="b1",
                                     bufs=3, name="pc")
                        pc3 = pc[:].rearrange("p (g t) -> p g t", t=TL)
                        cb = Cb2[:].unsqueeze(1).to_broadcast((128, NB, W))
                        nc.vector.tensor_tensor(
                            pc3[:, :, 0:k], r3(pq_t[:])[:, :, 0:k],
                            cb[:, :, 0:k], AOP.mult)
                        wc = bp.tile([128, NB * TL], BF16, tag="b1",
                                     bufs=3, name="wc")
                        for g in range(NB):
                            nc.vector.tensor_scalar_mul(
                                wc[:, g * TL:g * TL + k],
                                pc[:, g * TL:g * TL + k],
                                hin[:, s * NB + g:s * NB + g + 1])
                        if s == 0:
                            for b in range(6):
                                nc.tensor.matmul(
                                    psy[:, b * 512:(b + 1) * 512],
                                    identB_sb[:],
                                    wc[:, b * 512:(b + 1) * 512],
                                    start=False, stop=True)
                        else:
                            for g in range(NB):
                                nc.tensor.matmul(
                                    psy[:, g * TL:g * TL + k], identB_sb[:],
                                    wc[:, g * TL:g * TL + k],
                                    start=False, stop=False)
                        if s in (1, 2):
                            nc.tensor.matmul(
                                phb[:], identB_sb[:],
                                wc[:].rearrange("p (g t) -> p g t", t=TL)
                                [:, :, TL - 1],
                                start=False, stop=False)
                        if 0 < s < D_STATE - 1:
                            k2 = kof(s + 1)
                            pq2 = bp.tile([128, PW], BF16, tag="dA", bufs=3,
                                          name="pq2")
                            nc.vector.tensor_tensor(
                                r3(pq2[:])[:, :, 0:k2],
                                r3(pq_t[:])[:, :, 0:k2],
                                r3(qq[:])[:, :, 0:k2], AOP.mult)
                            return pq2
                        return pq_t

                    # chain pq upward for s=1..7 (a-half)
                    pq2_1 = bp.tile([128, PW], BF16, tag="dA", bufs=3,
                                    name="pq2_1")
                    nc.vector.tensor_tensor(
                        r3(pq2_1[:])[:, :, 0:kof(1)],
                        r3(qq[:])[:, :, 0:kof(1)],
                        r3(qq[:])[:, :, 0:kof(1)], AOP.mult)
                    pq = pq2_1
                    for s in range(1, 6):
                        pq = corr(s, pq)

                    # s=0 boundary contribution (full path runs later), then
                    # assemble y boundary column and launch AG2 early
                    Cb0 = bcast_row(64, "Cbc", 3)
                    wc0b = sp.tile([128, NB], BF16, tag="wc0b", bufs=1,
                                   name="wc0b")
                    nc.vector.tensor_tensor(
                        wc0b[:], r3(qq[:])[:, :, TL - 1],
                        Cb0[:, TL - 1:TL].to_broadcast((128, NB)), AOP.mult)
                    nc.vector.tensor_tensor(
                        wc0b[:], wc0b[:], hin[:, 0:NB], AOP.mult)
                    nc.tensor.matmul(phb[:], identB_sb[:], wc0b[:],
                                     start=False, stop=True)
                    ydb = sp.tile([128, NB], F32, tag="ydb", bufs=1,
                                  name="ydb")
                    nc.vector.tensor_tensor(
                        ydb[:], r3(us[:].bitcast(F32))[:, :, TL - 1],
                        dsk[:], AOP.mult)
                    nc.vector.tensor_tensor(ydb[:], ydb[:], phb[:], AOP.add)
                    nc.vector.tensor_tensor(
                        ydb[:], ydb[:], r3(sz[:])[:, :, TL - 1], AOP.mult)
                    ag2_in = dp.tile([128, NB], F32, tag="ag2i")
                    ag2_out = dp.tile([NC * 128, NB], F32, tag="ag2o")
                    nc.sync.dma_start(ag2_in[:], ydb[:])
                    nc.gpsimd.collective_compute(
                        "AllGather", AOP.bypass,
                        ins=[ag2_in[:].opt()], outs=[ag2_out[:].opt()],
                        replica_groups=[list(range(NC))])
                    agv2 = ag2_out[:].rearrange("(j p) g -> p j g", p=128)
                    yb_st = sp.tile([128, 96], F32, tag="qe_st", bufs=1,
                                    name="yb_st")
                    nc.sync.dma_start(
                        yb_st[:].rearrange("p (j g) -> p j g", g=NB), agv2)
                    yb_all = sp.tile([128, 96], wdt, tag="yb_all", bufs=1,
                                     name="yb_all")
                    nc.vector.tensor_copy(
                        yb_all[:].rearrange("p (g j) -> p g j", j=8),
                        yb_st[:].rearrange("p (j g) -> p g j", g=NB))

                    # ---- AG1b processing + corr s=8..15 ----
                    agbv = ag1b_out[:].rearrange("(j p) c -> p j c", p=128)
                    he_stb = sp.tile([128, 8 * 120], BF16, tag="he_stb",
                                     bufs=1, name="he_stb")
                    nc.sync.dma_start(
                        he_stb[:].rearrange("p (j sg) -> p j sg", j=8),
                        agbv[:, :, 0:120])
                    he_b = sp.tile([128, 8 * 120], F32, tag="heb", bufs=1,
                                   name="he_b")
                    nc.vector.tensor_copy(
                        he_b[:].rearrange("p (sg j) -> p sg j", j=8),
                        he_stb[:].rearrange("p (j sg) -> p sg j", j=8))
                    for s in range(6, D_STATE):
                        qp = combine(s, he_b, s - 6, qp)
                    for s in range(6, D_STATE):
                        pq = corr(s, pq)
                    corr(0, qq)  # full width, carries stop=True

                    # ---- y assembly ----
                    yd = bp.tile([128, NB * TL], F32, tag="u_pre", bufs=1,
                                 name="yd")
                    yd3 = yd[:].rearrange("p (g t) -> p g t", t=TL)
                    for g in range(NB):
                        nc.vector.tensor_scalar_mul(
                            yd3[:, g, :],
                            us[:, g * W:g * W + TL].bitcast(F32),
                            dsk[:, g:g + 1])
                    nc.vector.tensor_tensor(yd[:], yd[:], psy[:], AOP.add)
                    yg = bp.tile([128, PW], wdt, tag="ygsh", bufs=1,
                                 name="yg")
                    nc.vector.memset(
                        r3(yg[:].bitcast(F32) if li < 2
                           else yg[:])[:, :, W - 1], 0.0)
                    nc.gpsimd.tensor_tensor(
                        r3(yg[:])[:, :, 0:TL],
                        yd[:].rearrange("p (g t) -> p g t", t=TL),
                        r3(sz[:])[:, :, 0:TL], AOP.mult)

                    # ---- out_proj ----
                    xo = bp.tile([128, PWD], F32, tag="mcur", bufs=1,
                                 name="xo")
                    oprs = pp1.tile([128, NB * TL], F32, tag="bigps",
                                    name="oprs")
                    ph2 = oprs[:, 7 * TL:7 * TL + 48]
                    zero_banks(nc, identB_sb, zro512, oprs, 6)
                    for kb in range(NB):
                        wot = wp.tile([128, 768], wdt, tag="wmat",
                                      bufs=5, name="wot")
                        nc.scalar.dma_start(
                            wot[:],
                            out_projT[l, kb] if li < 2
                            else out_projTb[l - 2, kb])
                        for m in range(6):
                            nc.tensor.matmul(
                                oprs[:, m * TL:(m + 1) * TL],
                                wot[:, m * 128:(m + 1) * 128],
                                yg[:, kb * W:kb * W + TL],
                                start=False, stop=(kb == NB - 1))
                            nc.tensor.matmul(
                                ph2[:, m * 8:(m + 1) * 8],
                                wot[:, m * 128:(m + 1) * 128],
                                yb_all[:, kb * 8:(kb + 1) * 8],
                                start=False, stop=(kb == NB - 1))
                    for m in range(6):
                        nc.scalar.copy(xo[:, m * W:m * W + TL],
                                       oprs[:, m * TL:(m + 1) * TL])
                    ph2s = sp.tile([128, 48], F32, tag="ph2s", bufs=1,
                                   name="ph2s")
                    nc.scalar.copy(ph2s[:], ph2[:])
                    zero_ps(nc, oprs[0:8, 6 * TL:6 * TL + 768])
                    for m in range(6):
                        pht = oprs[0:8, 6 * TL + m * 128:
                                    6 * TL + (m + 1) * 128]
                        nc.tensor.matmul(
                            pht, ph2s[:, m * 8:(m + 1) * 8], identF_sb[:],
                            is_transpose=True, start=False, stop=True)
                        nc.scalar.copy(hstk[:, m * 128:(m + 1) * 128],
                                       pht)

                    # ---- attention prep (no AG2 dependency) ----
                    ps_sc = pp2.tile([8, TL], F32, tag="pss", name="ps_sc")
                    for g in range(NBD):
                        nc.tensor.matmul(
                            ps_sc[:], mst[:, g * 8:(g + 1) * 8],
                            ur[:, g * W:g * W + TL],
                            start=(g == 0), stop=(g == NBD - 1))
                    exps8 = sp.tile([8, TL], F32, tag="exp9", bufs=2,
                                    name="exps8")
                    nc.scalar.activation(exps8[:], ps_sc[:], AF.Exp,
                                         scale=SCALE)
                    expc = sp.tile([1, TL], F32, tag="exp9", bufs=2,
                                   name="expc")
                    nc.scalar.activation(expc[:], s_cur[:], AF.Exp,
                                         scale=SCALE)
                    expm = exps8
                    nc.vector.tensor_scalar_mul(expm[:], exps8[:],
                                                mask8_sb[:])
                    ps_den = pp2.tile([1, TL], F32, tag="pss", name="ps_den")
                    nc.tensor.matmul(ps_den[:], ones128_sb[0:8, :], expm[:],
                                     start=True, stop=False)
                    nc.tensor.matmul(ps_den[:], ones128_sb[0:1, :], expc[:],
                                     start=False, stop=True)
                    recip = sp.tile([1, TL], F32, tag="recip", bufs=1)
                    nc.vector.reciprocal(recip[:], ps_den[:])
                    e8b = sp.tile([128, TL], F32, tag="bc256", name="e8b")
                    nc.gpsimd.partition_broadcast(e8b[:], expc[:])
                    rcb = sp.tile([128, TL], F32, tag="bc256", name="rcb")
                    nc.gpsimd.partition_broadcast(rcb[:], recip[:])

                    # ---- GRM attention (hstk-dependent tail) ----
                    t1 = bp.tile([128, PWD], F32, tag="cs", bufs=1,
                                 name="t1")
                    e8v = e8b[:].unsqueeze(1).to_broadcast((128, NBD, TL))
                    nc.vector.tensor_tensor(
                        r3(t1[:])[:, :, 0:TL], r3(xo[:])[:, :, 0:TL], e8v,
                        AOP.mult)
                    t2 = bp.tile([128, PWD], F32, tag="prod", bufs=1,
                                 name="t2")
                    pshn = pp1.tile([128, NB * TL], F32, tag="bigps",
                                    name="pshn")
                    zero_banks(nc, identB_sb, zro512, pshn, 3)
                    for g in range(NBD):
                        nc.tensor.matmul(
                            pshn[:, g * TL:(g + 1) * TL],
                            hstk[:, g * 128:(g + 1) * 128],
                            expm[:], start=False, stop=True)
                        nc.vector.tensor_tensor(
                            t2[:, g * W:g * W + TL], t1[:, g * W:g * W + TL],
                            pshn[:, g * TL:(g + 1) * TL], AOP.add)
                    rcv = rcb[:].unsqueeze(1).to_broadcast((128, NBD, TL))
                    nc.gpsimd.tensor_tensor(
                        r3(xnxt[:])[:, :, 0:TL],
                        r3(t2[:])[:, :, 0:TL], rcv, AOP.mult)

                    if DEBUG:
                        dbg_x = nc.declare_dram_parameter(
                            f"dbg_x{li}", [128, PWD],
                            F32 if li == 0 else BF16, isOutput=True)
                        nc.sync.dma_start(
                            dbg_x[:],
                            xnxt[:].bitcast(F32) if li == 0 else xnxt[:])
                        if li == 1:
                            for nm, t in [("us1", us[:].bitcast(F32)),
                                          ("upre1", u_pre[:]),
                                          ("halo1", halo36[:]),
                                          ("hin1", hin[:]),
                                          ("yg1", yg[:].bitcast(F32)),
                                          ("xo1", xo[:]),
                                          ("hstk1", hstk[:]),
                                          ("ydb1", ydb[:])]:
                                dbg = nc.declare_dram_parameter(
                                    f"dbg_{nm}", list(t.shape), t.dtype,
                                    isOutput=True)
                                nc.sync.dma_start(dbg[:], t)
                        if li == 0:
                            for nm, t in [("us", us[:].bitcast(F32)),
                                          ("sz", sz[:]),
                                          ("rr", rr[:]),
                                          ("du", du[:]),
                                          ("qq", qq[:]),
                                          ("hin", hin[:]),
                                          ("yg", yg[:].bitcast(F32)),
                                          ("xo", xo[:]),
                                          ("ur", ur[:]),
                                          ("mst", mst[:]),
                                          ("hstk", hstk[:]),
                                          ("ydb", ydb[:]),
                                          ("upre", u_pre[:])]:
                                dt_ = t.dtype
                                dbg = nc.declare_dram_parameter(
                                    f"dbg_{nm}", list(t.shape), dt_,
                                    isOutput=True)
                                nc.sync.dma_start(dbg[:], t)
                    xcur = xnxt

            # ---------- head ----------
            xfin = xcur        # last layer's xnxt (bf16)
            xb = du[:, 0:PWD]
            nc.vector.tensor_copy(xb, xfin[:])
            hps = pp1.tile([128, NB * TL], F32, tag="bigps", name="hps")
            for quad in range(8):           # 8 x 8 row-blocks
                ost = bp.tile([128, 2048], F32, tag="u_pre", bufs=1,
                              name=f"ost{quad}")
                zero_banks(nc, identB_sb, zro512, hps, 4)
                for kb in range(NBD):
                    hw = wp.tile([128, 1024], BF16, tag="whd", bufs=3,
                                 name="hw")
                    nc.scalar.dma_start(
                        hw[:], headT[kb][:, quad * 1024:(quad + 1) * 1024])
                    for m in range(8):
                        nc.tensor.matmul(
                            hps[:, m * TL:(m + 1) * TL],
                            hw[:, m * 128:(m + 1) * 128],
                            xb[:, kb * W:kb * W + TL],
                            start=False, stop=(kb == NBD - 1))
                for m in range(8):
                    if m % 2 == 0:
                        nc.scalar.copy(ost[:, m * 256:(m + 1) * 256],
                                       hps[:, m * TL:(m + 1) * TL])
                    else:
                        nc.vector.tensor_copy(ost[:, m * 256:(m + 1) * 256],
                                              hps[:, m * TL:(m + 1) * TL])
                nc.sync.dma_start(
                    logits_out[quad * 1024:(quad + 1) * 1024, :].rearrange(
                        "(b p) t -> p b t", p=128),
                    ost[:].rearrange("p (b t) -> p b t", t=TL))

    nc.finalize()
    return nc


# ============================================================
# host side
# ============================================================

def _prep_shared(inputs):
    f = np.float32
    embed_w = np.ascontiguousarray(inputs["embed_w"], f)
    in_proj = np.ascontiguousarray(inputs["in_proj"], f)
    out_proj = np.ascontiguousarray(inputs["out_proj"], f)
    x_proj = np.ascontiguousarray(inputs["x_proj"], f)
    dt_w = np.ascontiguousarray(inputs["dt_w"], f)
    W_u = np.ascontiguousarray(inputs["W_u"], f)
    head_w = np.ascontiguousarray(inputs["head_w"], f)
    conv_w = np.ascontiguousarray(inputs["conv_w"], f)
    conv_b = np.ascontiguousarray(inputs["conv_b"], f)
    dt_b = np.ascontiguousarray(inputs["dt_b"], f)
    D_skip = np.ascontiguousarray(inputs["D_skip"], f)

    d = {}
    import ml_dtypes
    ipt = in_proj.reshape(L, 2, D_IN, D).transpose(0, 1, 3, 2)
    d["in_projT"] = np.ascontiguousarray(ipt.reshape(L, 2, NBD, 128, D_IN))
    d["in_projTb"] = d["in_projT"][2:].astype(ml_dtypes.bfloat16)
    opt = out_proj.transpose(0, 2, 1)
    d["out_projT"] = np.ascontiguousarray(opt.reshape(L, NB, 128, D))
    d["out_projTb"] = d["out_projT"][2:].astype(ml_dtypes.bfloat16)
    xpt = x_proj.transpose(0, 2, 1)
    d["x_projT"] = np.ascontiguousarray(xpt.reshape(L, NB, 128, 80))
    import ml_dtypes
    d["dt_wT"] = np.ascontiguousarray(
        dt_w.transpose(0, 2, 1)).astype(ml_dtypes.bfloat16)
    wut = W_u.transpose(0, 2, 1)
    d["W_uT"] = np.ascontiguousarray(wut.reshape(L, NBD, 128, D))
    d["W_uTb"] = d["W_uT"][2:].astype(ml_dtypes.bfloat16)
    hwt = head_w.reshape(K * V, D).T
    d["headT"] = np.ascontiguousarray(
        hwt.reshape(NBD, 128, K * V)).astype(ml_dtypes.bfloat16)
    cw = conv_w.reshape(L, NB, 128, D_CONV)
    d["convw_a"] = np.ascontiguousarray(cw.transpose(0, 3, 2, 1))
    d["convb_a"] = np.ascontiguousarray(
        conv_b.reshape(L, NB, 128).transpose(0, 2, 1))
    d["dtb_a"] = np.ascontiguousarray(
        -dt_b.reshape(L, NB, 128).transpose(0, 2, 1))
    d["dskip_a"] = np.ascontiguousarray(
        D_skip.reshape(L, NB, 128).transpose(0, 2, 1))
    ic = np.zeros((128, W), f)
    ic[:, :TL] = 1.0 / np.arange(1, TL + 1, dtype=f)[None, :]
    d["invcnt"] = ic
    d["ones128"] = np.ones((128, 1), f)
    d["identB"] = np.eye(128, dtype=f).astype(ml_dtypes.bfloat16)
    d["identF"] = np.eye(128, dtype=f)

    # embedding (host) + layer-0 conv halo
    tokens = np.asarray(inputs["tokens"])[0]                 # (T, K)
    x0 = embed_w[np.arange(K)[None, :], tokens].sum(axis=1)  # (T, D)
    d["_x0_full"] = x0.astype(f)
    upre = x0 @ in_proj[0, :D_IN].T                          # (T, D_IN)
    d["_upre_full"] = upre.astype(f)
    return d


def _prep_core(inputs, shared, c):
    f = np.float32
    x0 = shared["_x0_full"]
    x0c = np.zeros((128, NBD, W), f)
    seg = x0[c * TL:(c + 1) * TL]                            # (TL, D)
    x0c[:, :, :TL] = seg.T.reshape(NBD, 128, TL).transpose(1, 0, 2)
    h0 = np.zeros((128, NB, 3), f)
    if c > 0:
        hseg = shared["_upre_full"][c * TL - 3:c * TL]       # (3, D_IN)
        h0[:] = hseg.T.reshape(NB, 128, 3).transpose(1, 0, 2)
    import ml_dtypes
    gsel = np.zeros((128, 8 * 36), ml_dtypes.bfloat16)
    if c > 0:
        gsel[:, (c - 1) * 36:c * 36] = 1.0
    m8 = np.zeros((8, 1), f)
    m8[:c] = 1.0
    s96 = np.zeros((128, NB, 8), f)
    if c > 0:
        s96[:, :, c - 1] = 1.0
    return dict(x0_in=x0c.reshape(128, NBD * W),
                halo0=h0.reshape(128, NB * 3),
                gsel_h=gsel, mask8=m8,
                sel96=s96.reshape(128, 96))


def make_in_maps(inputs):
    shared = _prep_shared(inputs)
    per_core = [dict(_prep_core(inputs, shared, c)) for c in range(NC)]
    shared_clean = {k: v for k, v in shared.items()
                    if not k.startswith("_")}
    return [dict(shared_clean, **pc) for pc in per_core]


_BUILT = {}


def _get_runner(repeat=1):
    if repeat in _BUILT:
        return _BUILT[repeat]
    import jax
    from jax.sharding import Mesh, PartitionSpec
    from jax.experimental.shard_map import shard_map
    from concourse.bass2jax import (
        _bass_exec_p, install_neuronx_cc_hook, partition_id_tensor)

    nc = build_module(repeat)
    install_neuronx_cc_hook()

    partition_name = (
        nc.partition_id_tensor.name if nc.partition_id_tensor else None)
    in_names, out_names, out_avals, zero_outs = [], [], [], []
    for alloc in nc.m.functions[0].allocations:
        if not isinstance(alloc, mybir.MemoryLocationSet):
            continue
        name = alloc.memorylocations[0].name
        if alloc.kind == "ExternalInput":
            if name != partition_name:
                in_names.append(name)
        elif alloc.kind == "ExternalOutput":
            shape = tuple(alloc.tensor_shape)
            dtype = mybir.dt.np(alloc.dtype)
            out_names.append(name)
            out_avals.append(jax.core.ShapedArray(shape, dtype))
            zero_outs.append(np.zeros(shape, dtype))
    n_params, n_outs = len(in_names), len(out_avals)
    all_in = list(in_names) + list(out_names)
    if partition_name is not None:
        all_in.append(partition_name)

    def _body(*args):
        operands = list(args)
        if partition_name is not None:
            operands.append(partition_id_tensor())
        return tuple(_bass_exec_p.bind(
            *operands, out_avals=tuple(out_avals), in_names=tuple(all_in),
            out_names=tuple(out_names), lowering_input_output_aliases=(),
            sim_require_finite=False, sim_require_nnan=False, nc=nc))

    donate = tuple(range(n_params, n_params + n_outs))
    devices = jax.devices()[:NC]
    mesh = Mesh(np.asarray(devices), ("core",))
    fn = jax.jit(
        shard_map(_body, mesh=mesh,
                  in_specs=(PartitionSpec("core"),) * (n_params + n_outs),
                  out_specs=(PartitionSpec("core"),) * n_outs,
                  check_rep=False),
        donate_argnums=donate, keep_unused=True)

    runner = dict(fn=fn, in_names=in_names, out_names=out_names,
                  out_avals=out_avals, zero_outs=zero_outs)
    _BUILT[repeat] = runner
    return runner


def run_spmd(inputs, repeat=1):
    import jax
    r = _get_runner(repeat)
    per_core = make_in_maps(inputs)
    concat_in = [
        np.concatenate([np.asarray(per_core[c][n]) for c in range(NC)],
                       axis=0)
        for n in r["in_names"]]
    concat_zero = [
        np.zeros((NC * z.shape[0], *z.shape[1:]), z.dtype)
        for z in r["zero_outs"]]
    outs = r["fn"](*concat_in, *concat_zero)
    jax.block_until_ready(outs)
    logits = np.asarray(outs[0]).reshape(NC, K * V, TL)
    out = logits.transpose(0, 2, 1).reshape(1, T, K, V)
    return out


def kernel(**inputs):
    return run_spmd(inputs, repeat=1)
